# revision 7
# baseline (speedup 1.0000x reference)
"""Chamfer loss kernel for Trainium2 (8 NeuronCores, SPMD data-parallel).

Problem: x, y of shape (2, 16, 1024, 3) fp32.
  dist[b, i, j] = sqrt(EPS + max(||x[b,j] - y[b,i]||^2, 0))  over (BT=32, N=1024, N)
  out = mean(min_i dist) + mean(min_j dist)   (a scalar)

Strategy:
  - Shard the fused BT=32 batch dim across 8 cores (4 batch elements/core).
  - Squared distances via a K=13 augmented bf16 matmul: 2-way hi/mid split of
    the K=5 augmented form [p2, 1, -2p] . [1, q2, q], keeping the (hi,hi),
    (hi,mid), (mid,hi) cross terms. bf16 products are exact in fp32 PSUM;
    ~1e-4 abs noise on sq, ~9e-4 rel on the final scalar (gate is 2e-2).
  - 64 PSUM tiles [128, 1024] per core; the min-reduction is the wall: only
    ACT (0.83 ns/col) and DVE (1.04 ns/col) can read PSUM, so the floor is
    ~658 ns/tile on DVE with ACT copying half of every tile.
  - Per tile: ACT copies the h1 half to SBUF bf16 (612 ns) while DVE runs a
    tensor_tensor_scan(min,min) pairing the h0 PSUM half against that copy
    (658 ns). The scan output uses a STRIDE-0 destination AP: all 512
    running-min values land on one column, so the final write IS the row
    min - no lane buffers and no extract pass (device-verified exact).
  - The copy is ordered after the full psum tile (add_dep) so the scan's PE
    wait prunes transitively: the scan opcode only has one sync-wait slot in
    walrus codegen.
  - Tile 0 is a DVE solo tensor_reduce (no ACT dependency) so DVE starts
    ~0.8 us earlier; everything else pipelines at the DVE rate with zero
    mid-stream gaps (engine busy: DVE 42.6 us, ACT 38.6 us, PE 27.6 us).
  - min(sqrt(eps+max(s,0))) == sqrt(eps+max(min(s),0)) by monotonicity, so
    only the (BT, N) min values need the sqrt/clamp, done on host along with
    the final mean (the "cheap all-reduce").
  - The emitted sync graph is post-processed by _strip_redundant_waits (a
    transitive vector-clock closure) to fit walrus per-opcode sync-wait
    encoding budgets; a shared scratch output would otherwise chain DVE ops
    through same-engine WAW semaphores costing ~160 ns/tile.
"""

import os
import sys

import numpy as np

for _p in ("/opt/trn_rl_repo",):
    if os.path.isdir(_p) and _p not in sys.path:
        sys.path.insert(0, _p)

import concourse.bass as bass
import concourse.mybir as mybir
from concourse.bass_utils import run_bass_kernel_spmd
from concourse.tile import TileContext, add_dep_helper

EPS = 1e-6
B, T, N, D = 2, 16, 1024, 3
BT = B * T
NCORES = 8
BPC = BT // NCORES  # batch elements per core
KAUG = 13  # augmented contraction rows after 2-way bf16 split
NT = BPC * 2 * 8  # 64 tiles of [128, 1024] per core
CC = 80.0  # softmin sharpness: k_i = CC / mhat_i
MHAT_FLOOR = 1e-4
SUB = 128  # subset width for the mhat estimate

AUG_W = 4 * BPC * N  # st/mv for both orientations, batch-major

_F32 = mybir.dt.float32
_BF16 = mybir.dt.bfloat16
_MIN = mybir.AluOpType.min

# 16 ring cycles x 4 psum slots = 32 tile-PAIRS. "M" pair: ACT chunk-copies
# both h1 halves (1 op), DVE ttrs each h0 PSUM half against its copy.
# "F" pair: ACT chunk-copies both FULL tiles (1 op), DVE ttrs the two SBUF
# halves - cheaper DVE (594 vs 658) and frees the PSUM slots at copy time.
# 9 F pairs soak ACT's slack; spread evenly among the 32 pairs.
PAIR_SCHED = ["M"] * 32
SCHED = "".join(p * 2 for p in PAIR_SCHED)  # per-tile view (all reduce-mode)

_NC_CACHE = None


def _build_nc():
    """One NeuronCore program; identical on all 8 cores (inputs differ)."""
    nc = bass.Bass()

    aug = nc.dram_tensor("aug", [KAUG, AUG_W], _BF16, kind="ExternalInput")
    # outs col t: per-tile row-min (squared distances)
    outs_d = nc.dram_tensor("outs", [128, NT], _F32, kind="ExternalOutput")

    with TileContext(nc) as tc:
        with (
            tc.tile_pool(name="inp", bufs=1) as inp_pool,
            tc.tile_pool(name="outp", bufs=1) as out_pool,
            tc.tile_pool(name="scr", bufs=1) as scr_pool,
            tc.tile_pool(name="ps", bufs=1, space="PSUM") as ps_pool,
        ):
            aug_t = inp_pool.tile([KAUG, AUG_W], _BF16, name="aug_t")
            # constants + k-chain scratch on DVE so the ACT softmin's waits
            # collapse to a single DVE semaphore.
            outs_t = out_pool.tile([128, NT], _F32, name="outs_t")
            # h1-half copies for the scans (ring; one buffer per tile)
            h1s = [
                scr_pool.tile([128, 2, 512], _BF16, name=f"h1_{i}")
                for i in range(6)
            ]

            # PE warm-up into slot-2/3 region, overlapping the input DMA.
            warm_t = scr_pool.tile([32, 640], _BF16, name="warm_t")
            nc.gpsimd.memset(warm_t[:], 0.0)
            ps = ps_pool.tile([128, 4096], _F32, name="ps")  # whole PSUM
            for w in range(2):
                nc.tensor.matmul(
                    ps[:, 2048 + w * 512 : 2048 + (w + 1) * 512],
                    warm_t[:, 0:128],
                    warm_t[:, 128:640],
                    start=True,
                    stop=True,
                )

            # input load: first batch element in two small chunks (faster
            # first-tile latency), the rest one chunk per batch element
            chunk_edges = [0, 2 * N, 4 * N, 8 * N, 12 * N, 16 * N]
            for c in range(len(chunk_edges) - 1):
                lo, hi = chunk_edges[c], chunk_edges[c + 1]
                nc.sync.dma_start(out=aug_t[:, lo:hi], in_=aug[:, lo:hi])

            def mm_half(t, slot, h):
                b, o, blk = t // 16, (t // 8) % 2, t % 8
                st_off = (b * 4 + 2 * o) * N
                mv_off = (b * 4 + 2 * o + 1) * N
                lhsT = aug_t[:, st_off + blk * 128 : st_off + blk * 128 + 128]
                return nc.tensor.matmul(
                    ps[:, slot * 1024 + h * 512 : slot * 1024 + (h + 1) * 512],
                    lhsT,
                    aug_t[:, mv_off + h * 512 : mv_off + h * 512 + 512],
                    start=True,
                    stop=True,
                )

            def scan_to_out(t, in0, in1):
                # stride-0 output: all 512 running-min values land on the
                # same column, so the final write IS the row min. Saves the
                # lane buffers and the strided extract pass entirely.
                dst = outs_t[:, t : t + 1].to_broadcast((128, 512))
                nc.vector.tensor_tensor_scan(dst, in0, in1, 3.0e38, _MIN, _MIN)

            for pidx in range(32):
                t = 2 * pidx
                i = (2 * pidx) % 4  # slot of the pair's first tile
                h1 = h1s[pidx % len(h1s)]
                for q in range(2):
                    if t + q == 0:
                        # tile 0: DVE solo reduce; no ACT dependency, so the
                        # DVE starts ~0.8us earlier than the first scan could
                        mm_half(0, 0, 1)
                        mm_half(0, 0, 0)
                        continue
                    mm_half(t + q, i + q, 1)
                    cp = nc.scalar.copy(
                        h1[:, q, :],
                        ps[:, (i + q) * 1024 + 512 : (i + q + 1) * 1024],
                    )
                    mm0 = mm_half(t + q, i + q, 0)
                    # the copy waits the full psum tile so the scan's PE
                    # wait prunes transitively (scan opcode allows only
                    # one sync wait on walrus)
                    add_dep_helper(cp.ins, mm0.ins, sync=True,
                                   reason="cp waits full psum tile")
                if t == 0:
                    nc.vector.tensor_reduce(
                        outs_t[:, 0:1], ps[:, 0:1024],
                        op=_MIN, axis=mybir.AxisListType.XYZW,
                    )
                    scan_to_out(
                        1, ps[:, 1024:1536], h1[:, 1, :]
                    )
                else:
                    for q in range(2):
                        scan_to_out(
                            t + q,
                            ps[:, (i + q) * 1024 : (i + q) * 1024 + 512],
                            h1[:, q, :],
                        )

            # split output DMA: bulk early (overlapped), tiny final
            nc.sync.dma_start(out=outs_d[:, 0:56], in_=outs_t[:, 0:56])
            nc.sync.dma_start(out=outs_d[:, 56:64], in_=outs_t[:, 56:64])

    return _strip_redundant_waits(nc)


def _strip_redundant_waits(nc):
    """Transitive vector-clock closure over the emitted sync graph; drops
    every semaphore wait whose condition is already implied at the waiting
    instruction's dispatch point.

    Soundness: semaphores only increase; each engine/DMA queue dispatches and
    completes its instructions in program order (PE completion is pc-monotone
    per the HW docs; DVE/ACT are serial with a pipeline drain between ops).
    So (a) an instruction inherits everything instructions earlier on its own
    engine acquired via their waits, and for serial engines also everything
    published by their completions, and (b) waiting `sem >= v` also conveys
    the dispatch-knowledge of the instruction whose completion brought `sem`
    to `v` (plus, by in-order completion, of all earlier instructions on that
    engine). Tile emits waits per-processor without this closure, which
    overflows the per-opcode sync-wait encoding budget (walrus "Too many
    sync wait commands")."""

    def merge(dst, srcd):
        for s, v in srcd.items():
            if dst.get(s, -1) < v:
                dst[s] = v

    cum = {}  # sem id -> cumulative inc value so far
    poisoned = set()  # sems with non-inc updates: no pruning
    publishes = {}  # sem id -> list of (value, knowledge dict), ascending
    know = {}  # engine -> dispatch knowledge {sem: value}
    done_know = {}  # engine -> completion knowledge union of all its insts
    for bb in nc.m.functions[0].blocks:
        for inst in bb.instructions:
            si = inst.sync_info
            if si is None:
                continue
            e = inst.engine
            k = know.setdefault(e, {})
            dk = done_know.setdefault(e, {})
            if e in (mybir.EngineType.DVE, mybir.EngineType.Activation):
                merge(k, dk)
            ws = si.on_wait or []

            def absorbed(base, waits):
                kk = dict(base)
                for w2 in waits:
                    v2 = w2.wait_value or 0
                    if kk.get(w2.id, -1) < v2:
                        kk[w2.id] = v2
                    for pv, pk in publishes.get(w2.id, ()):
                        if pv <= kk.get(w2.id, -1):
                            merge(kk, pk)
                return kk

            prunable = [
                w
                for w in ws
                if w.sync_type == "semaphore"
                and w.wait_mode == "sem-ge-imm"
                and w.wait_reg is None
                and w.id not in poisoned
            ]
            fixed = [w for w in ws if w not in prunable]
            kept = list(prunable)
            changed = True
            while changed:
                changed = False
                for w in list(kept):
                    others = [x for x in kept if x is not w] + fixed
                    if absorbed(k, others).get(w.id, -1) >= (w.wait_value or 0):
                        kept.remove(w)
                        changed = True
            k.update(absorbed(k, ws))
            if len(kept) + len(fixed) != len(ws):
                si.on_wait = fixed + kept
            ups = [
                u
                for u in (si.on_update or [])
                if u.sync_type == "semaphore"
            ]
            bad = [u for u in ups if u.update_mode not in ("sem-inc", "sem-add-imm")]
            for u in bad:
                poisoned.add(u.id)
                publishes.pop(u.id, None)
            ups = [u for u in ups if u.update_mode in ("sem-inc", "sem-add-imm")]
            if ups:
                snap = dict(dk)
                merge(snap, k)
                for u in ups:
                    cum[u.id] = cum.get(u.id, 0) + (u.update_value or 0)
                for u in ups:
                    snap[u.id] = max(snap.get(u.id, -1), cum[u.id])
                for u in ups:
                    if u.id not in poisoned:
                        publishes.setdefault(u.id, []).append((cum[u.id], snap))
                merge(dk, snap)
            else:
                merge(dk, k)
    return nc


def _get_nc():
    global _NC_CACHE
    if _NC_CACHE is None:
        _NC_CACHE = _build_nc()
    return _NC_CACHE


try:
    from ml_dtypes import bfloat16 as _np_bf16
except ImportError:
    raise RuntimeError("ml_dtypes required for bf16 host-side splitting")


def _split2(a):
    hi = a.astype(_np_bf16)
    mid = (a - hi.astype(np.float32)).astype(_np_bf16)
    return hi.astype(np.float32), mid.astype(np.float32)


def _aug13(p, q):
    """Stationary points p, moving points q: (N,3) each ->
    st13, mv13 of shape (13, N) whose column-dot equals ||p_i - q_j||^2
    up to the 2-way bf16 split truncation."""
    p2 = np.einsum("nd,nd->n", p, p).astype(np.float32)
    q2 = np.einsum("nd,nd->n", q, q).astype(np.float32)
    m2p = (-2.0 * p).T.astype(np.float32)  # (3, N)
    qt = q.T.astype(np.float32)
    p2h, p2m = _split2(p2)
    q2h, q2m = _split2(q2)
    mph, mpm = _split2(m2p)
    qh, qm = _split2(qt)
    one = np.ones_like(p2)
    st = np.empty((KAUG, len(p)), np.float32)
    mv = np.empty((KAUG, len(q)), np.float32)
    st[0], mv[0] = p2h, one
    st[1], mv[1] = one, q2h
    st[2:5], mv[2:5] = mph, qh
    st[5], mv[5] = one, q2m
    st[6:9], mv[6:9] = mph, qm
    st[9], mv[9] = p2m, one
    st[10:13], mv[10:13] = mpm, qh
    return st, mv


def _run_device(x, y, **kw):
    xf = np.asarray(x, dtype=np.float32).reshape(BT, N, D)
    yf = np.asarray(y, dtype=np.float32).reshape(BT, N, D)

    in_maps = []
    for c in range(NCORES):
        aug = np.empty((KAUG, AUG_W), np.float32)
        for b in range(BPC):
            g = c * BPC + b
            # orientation 0: rows = y points; orientation 1: rows = x points
            stA, mvA = _aug13(yf[g], xf[g])
            stB, mvB = _aug13(xf[g], yf[g])
            base = b * 4 * N
            aug[:, base : base + N] = stA
            aug[:, base + N : base + 2 * N] = mvA
            aug[:, base + 2 * N : base + 3 * N] = stB
            aug[:, base + 3 * N : base + 4 * N] = mvB
        in_maps.append({"aug": np.ascontiguousarray(aug.astype(_np_bf16))})

    return run_bass_kernel_spmd(_get_nc(), in_maps, list(range(NCORES)), **kw)


def _decode(outs):
    """outs: (128, 64) device tile -> (NT, 128) per-tile sq-mins."""
    return np.ascontiguousarray(outs.T.astype(np.float32))


def kernel(x, y):
    res = _run_device(x, y)
    sq = np.concatenate(
        [_decode(res.results[c]["outs"]) for c in range(NCORES)], axis=0
    )  # (512, 128): every (core, tile) covers 128 points of one orientation
    d = np.sqrt(EPS + np.maximum(sq, 0.0), dtype=np.float32)
    o_mask = np.array([((t // 8) % 2) for t in range(NT)] * NCORES)
    dA = d[o_mask == 0]  # rows = y points: min over x
    dB = d[o_mask == 1]  # rows = x points: min over y
    out = dA.mean(dtype=np.float64) + dB.mean(dtype=np.float64)
    return np.float32(out)
